# revision 1
# baseline (speedup 1.0000x reference)
"""Multi-head attention (B=4, L=2048, C=1024, H=16, D=64) on 8 TRN2 NeuronCores.

Sharding: core c handles batch b = c//2 and head-group hg = c%2 (8 heads).
Megatron-style: w_qkv column-sharded, w_proj row-sharded; the proj all-reduce
(2 cores per batch) happens on the host during unshard.

Per-core dataflow (all matmuls fp32r = full PE rate, ~1e-4 rel err):
  - host passes x[b] TRANSPOSED (xT [c, l]) plus pre-transposed/permuted
    weight slices, so the device needs zero transposes:
      q/k proj:  qT[f, l]  = (wqkT chunk).T @ xT     (f on partitions)
      v proj:    V[l, fv]  = (xT chunk).T @ wvT      (natural, for attn@V lhsT)
      scores:    S^T[k, q] = kT.T @ qT   (contraction d=64; two heads packed
                 per 128 partitions, row-tiled via tile_position)
      softmax:   exp on ScalarE straight out of PSUM (scale=1/8 fused);
                 no max-subtraction needed (|scores| <~ 6)
      attn@V:    O^T[d, q] = V'.T @ P^T accumulated over 16 k-chunks, with a
                 ones column in V' producing the softmax denominator in row 64
      norm:      reciprocal + gpsimd partition_broadcast + fused mul (DVE)
      proj:      out[l, co] = (O^T chunk).T @ wpT, partial over 512 dims
  - RoPE: w_qkv rows for q/k are host-permuted so that within each
    32-partition quadrant, even-d lanes sit at rows 0:16 and odd-d at 16:32.
    Then rope = qps*cos4 + quadrant_swap(qps)*sin4s, where the swap is a
    single DVE stream_shuffle and cos4/sin4s are host-built broadcast tables
    (sign folded into sin4s).
"""

import sys

sys.path.insert(0, "/opt/trn_rl_repo")

import numpy as np

B, L, C, H, D = 4, 2048, 1024, 16, 64
NCORES = 8
QT = 512          # q-tile; score mm N=512 = one full PSUM bank (HW requires one tile_position mm per bank)
GRP = 2           # score chunk-banks per exp group
PSSC_BUFS = 2
PT_BUFS = 4
QK_BUFS = 3
OTP_BUFS = 2
NORM = "dve"      # softmax-denominator broadcast impl: dve | gpsimd | none
PSQP_BUFS = 2
ABLATE = set()    # debug: subset of {"exp", "rope", "av", "interleave", "phasec"}
_built = {}


def _build(nc_mod):
    """Build the per-core Bass program (identical on all cores)."""
    import concourse.mybir as mybir
    import concourse.tile as tile
    from concourse import bacc
    from concourse.alu_op_type import AluOpType

    F32 = mybir.dt.float32
    F32R = mybir.dt.float32r
    EXP = mybir.ActivationFunctionType.Exp
    MULT = AluOpType.mult
    ADD = AluOpType.add
    BYPASS = AluOpType.bypass

    NKC = C // 128          # 8 contraction chunks for qkv proj
    NLT = L // 128          # 16 l-tiles (V rows, proj rows, k-chunks)
    NQT = L // QT           # q-tiles per pair
    NPAIR = 4               # head pairs per core
    FV = 512                # v features per core
    VW = 65                 # V columns incl. ones

    nc = bacc.Bacc(None, target_bir_lowering=False)

    xT_d = nc.dram_tensor("xT", [128, NKC, L], F32R, kind="ExternalInput")
    wqkT_d = nc.dram_tensor("wqkT", [8, 128, NKC, 128], F32R, kind="ExternalInput")
    wvT_d = nc.dram_tensor("wvT", [NKC, 128, FV], F32R, kind="ExternalInput")
    wpT_d = nc.dram_tensor("wpT", [128, NPAIR, C], F32R, kind="ExternalInput")
    cos4_d = nc.dram_tensor("cos4", [128, L], F32, kind="ExternalInput")
    sin4_d = nc.dram_tensor("sin4", [128, L], F32, kind="ExternalInput")
    outp_d = nc.dram_tensor("outp", [NLT, 128, C], F32, kind="ExternalOutput")

    SWAP_MASK = list(range(16, 32)) + list(range(16))

    with tile.TileContext(nc) as tc:
        import contextlib

        with contextlib.ExitStack() as outer:
            persist = outer.enter_context(tc.tile_pool(name="persist", bufs=1))
            qk_pool = outer.enter_context(tc.tile_pool(name="qkt", bufs=QK_BUFS))
            ot_pool = outer.enter_context(tc.tile_pool(name="otp", bufs=OTP_BUFS))
            dram = outer.enter_context(tc.tile_pool(name="dram", bufs=1, space="DRAM"))
            OT_dram = dram.tile([128, NPAIR, L], F32R)

            with contextlib.ExitStack() as mid:
                xpool = mid.enter_context(tc.tile_pool(name="xT", bufs=1))
                wvpool = mid.enter_context(tc.tile_pool(name="wV", bufs=1))
                wpool = mid.enter_context(tc.tile_pool(name="wA", bufs=2))
                cpool = mid.enter_context(tc.tile_pool(name="csn", bufs=1))
                tpool = mid.enter_context(tc.tile_pool(name="tmp", bufs=2))
                pt_pool = mid.enter_context(tc.tile_pool(name="pt", bufs=PT_BUFS))
                ps_sc = mid.enter_context(tc.tile_pool(name="ps_sc", bufs=PSSC_BUFS, space="PSUM"))
                ps_av = mid.enter_context(tc.tile_pool(name="ps_av", bufs=1, space="PSUM"))
                ps_qp = mid.enter_context(tc.tile_pool(name="ps_qp", bufs=PSQP_BUFS, space="PSUM"))

                # ---- persistent tensors ----
                V_t = persist.tile([128, NLT, 8, VW], F32R, tag="V")

                # ---- input DMAs (wvT first: the A1 matmuls need all of it) ----
                wvT_t = wvpool.tile([128, NKC, FV], F32R, tag="wv")
                xT_t = xpool.tile([128, NKC, L], F32R)
                nc.sync.dma_start(xT_t[:, :, 0:128], xT_d[:, :, 0:128])
                for kc in range(NKC):
                    nc.sync.dma_start(wvT_t[:, kc, :], wvT_d[kc])
                for lt in range(1, NLT):
                    sl = slice(lt * 128, (lt + 1) * 128)
                    nc.sync.dma_start(xT_t[:, :, sl], xT_d[:, :, sl])
                cos4_t = cpool.tile([128, L], F32)
                sin4_t = cpool.tile([128, L], F32)
                ones_t = cpool.tile([128, NLT, 8], F32)
                nc.vector.memset(ones_t[:], 1.0)
                nc.vector.tensor_copy(V_t[:, :, :, 64:65], ones_t[:, :, :, None])
                norm_scr = []
                for i in range(4):
                    nsc = cpool.tile([64, QT], F32, tag=f"nsc{i}")
                    nc.vector.memset(nsc[:], 1.0)
                    norm_scr.append(nsc)

                # ---- phase A1: V = x @ Wv (natural layout) ----
                for lt in range(NLT):
                    vps = ps_qp.tile([128, 512], F32, tag="qps")
                    for kc in range(NKC):
                        nc.tensor.matmul(
                            vps[:],
                            xT_t[:, kc, lt * 128:(lt + 1) * 128],
                            wvT_t[:, kc, :],
                            start=(kc == 0),
                            stop=(kc == NKC - 1),
                        )
                    nc.vector.tensor_copy(V_t[:, lt, :, 0:64], vps[:])

                nc.sync.dma_start(cos4_t[:], cos4_d[:])
                nc.sync.dma_start(sin4_t[:], sin4_d[:])

                # ---- phase A2 helper: qT/kT for one f-tile (one pair, q or k) ----
                qkT = {}

                def emit_qk(ft):
                    wqk = wpool.tile([128, NKC, 128], F32R, tag="wqk")
                    nc.sync.dma_start(wqk[:], wqkT_d[ft])
                    dst = qk_pool.tile([128, L], F32R, tag="qkt")
                    qkT[ft] = dst
                    for lq in range(L // 512):
                        qps = ps_qp.tile([128, 512], F32, tag="qps")
                        sl = slice(lq * 512, (lq + 1) * 512)
                        for kc in range(NKC):
                            nc.tensor.matmul(
                                qps[:],
                                wqk[:, kc, :],
                                xT_t[:, kc, sl],
                                start=(kc == 0),
                                stop=(kc == NKC - 1),
                            )
                        if "rope" in ABLATE:
                            nc.vector.tensor_copy(dst[:, sl], qps[:])
                        else:
                            # RoPE: dst = qps*cos4 + swap(qps)*sin4s
                            shuf = tpool.tile([128, 512], F32, tag="shuf")
                            nc.vector.stream_shuffle(shuf[:], qps[:], SWAP_MASK)
                            nc.vector.tensor_tensor(dst[:, sl], qps[:], cos4_t[:, sl], op=MULT)
                            nc.vector.tensor_tensor(shuf[:], shuf[:], sin4_t[:, sl], op=MULT)
                            nc.vector.tensor_tensor(dst[:, sl], dst[:, sl], shuf[:], op=ADD)

                emit_qk(0)
                emit_qk(4)

                if "only_a" in ABLATE:
                    nc.sync.dma_start(outp_d[0, :, 0:512].bitcast(F32R), qkT[0][:, 0:512])
                    nc.sync.dma_start(outp_d[1, :, 0:512].bitcast(F32R), qkT[4][:, 0:512])
                    for ft in [1, 5, 2, 6, 3, 7]:
                        emit_qk(ft)
                        nc.sync.dma_start(
                            outp_d[ft % NLT, :, 0:512].bitcast(F32R), qkT[ft][:, 0:512]
                        )

                # ---- phase B: attention per (pair, q-tile), A2 interleaved ----
                NGRP = 2 * NLT // GRP  # exp groups per pair-qtile
                for pr in range(NPAIR if "only_a" not in ABLATE else 0):
                    qT_t, kT_t = qkT[pr], qkT[4 + pr]
                    OT_t = ot_pool.tile([128, L], F32R, tag="otp")
                    for qt in range(NQT):
                        qsl = slice(qt * QT, (qt + 1) * QT)
                        avA = ps_av.tile([128, QT], F32, tag="avA")
                        avB = ps_av.tile([128, QT], F32, tag="avB")
                        av = [avA, avB]
                        for g0 in range(0, 2 * NLT, GRP):
                            glen = min(GRP, 2 * NLT - g0)
                            sc = ps_sc.tile([128, GRP, QT], F32, tag="sc")
                            pt = pt_pool.tile([128, GRP, QT], F32R, tag="pt")
                            for j in range(glen):
                                s = g0 + j
                                kt, hd = s // 2, s % 2
                                if "noscore" in ABLATE:
                                    nc.tensor.matmul(
                                        sc[:, j, :],
                                        kT_t[0:128, kt * 128:(kt + 1) * 128],
                                        qT_t[0:128, qsl],
                                        start=True,
                                        stop=True,
                                    )
                                    continue
                                nc.tensor.matmul(
                                    sc[:, j, :],
                                    kT_t[hd * 64:(hd + 1) * 64, kt * 128:(kt + 1) * 128],
                                    qT_t[hd * 64:(hd + 1) * 64, qsl],
                                    start=True,
                                    stop=True,
                                    tile_position=(hd * 64, 0),
                                )
                            if "exp" in ABLATE:
                                nc.vector.tensor_copy(pt[:, 0:glen, :], sc[:, 0:glen, :])
                            else:
                                nc.scalar.activation(pt[:, 0:glen, :], sc[:, 0:glen, :], EXP, scale=float(D) ** -0.5)
                            for j in range(glen):
                                s = g0 + j
                                kt, hd = s // 2, s % 2
                                if "av" in ABLATE:
                                    continue
                                nc.tensor.matmul(
                                    av[hd][0:VW, :],
                                    V_t[:, kt, pr * 2 + hd, :],
                                    pt[:, j, :],
                                    start=(kt == 0),
                                    stop=(kt == NLT - 1),
                                )
                        # normalize and write O^T
                        for hd in range(2):
                            if "av" in ABLATE:
                                nc.vector.tensor_copy(
                                    OT_t[hd * 64:(hd + 1) * 64, qsl], pt[0:64, 0, :]
                                )
                                continue
                            if NORM == "none":
                                nc.vector.tensor_copy(
                                    OT_t[hd * 64:(hd + 1) * 64, qsl], av[hd][0:64, :]
                                )
                                continue
                            if NORM == "gpsimd":
                                rd = tpool.tile([1, QT], F32, tag="rd")
                                nc.vector.reciprocal(rd[:], av[hd][64:65, :])
                                db = tpool.tile([64, QT], F32, tag="db")
                                nc.gpsimd.partition_broadcast(db[:], rd[:])
                            else:  # dve
                                rd = norm_scr[2 * hd]
                                nc.vector.reciprocal(rd[0:1, :], av[hd][64:65, :])
                                nc.vector.reciprocal(rd[32:33, :], av[hd][64:65, :])
                                db = norm_scr[2 * hd + 1]
                                nc.vector.stream_shuffle(db[:], rd[:], [0] * 32)
                            nc.vector.scalar_tensor_tensor(
                                OT_t[hd * 64:(hd + 1) * 64, qsl],
                                av[hd][0:64, :],
                                1.0,
                                db[:],
                                op0=MULT,
                                op1=MULT,
                            )
                        nc.sync.dma_start(OT_dram[:, pr, qsl], OT_t[:, qsl])
                        if pr < NPAIR - 1 and "interleave" not in ABLATE:
                            if qt == min(1, NQT - 1):
                                emit_qk(pr + 1)
                            if qt == min(3, NQT - 1):
                                emit_qk(5 + pr)
                    if pr < NPAIR - 1 and "interleave" in ABLATE:
                        emit_qk(pr + 1)
                        emit_qk(5 + pr)



            # ---- phase C: partial out-proj ----
            with contextlib.ExitStack() as cstack:
                ob_pool = cstack.enter_context(tc.tile_pool(name="ob", bufs=4))
                ps_c = cstack.enter_context(tc.tile_pool(name="ps_c", bufs=4, space="PSUM"))
                wp_pool = cstack.enter_context(tc.tile_pool(name="wp", bufs=1))
                otc_pool = cstack.enter_context(tc.tile_pool(name="otc", bufs=6))
                wpT_t = wp_pool.tile([128, NPAIR, C], F32R)
                for kd in range(NPAIR):
                    nc.sync.dma_start(wpT_t[:, kd, :], wpT_d[:, kd, :])
                for lt in range(NLT if "phasec" not in ABLATE else 0):
                    lsl = slice(lt * 128, (lt + 1) * 128)
                    ot_c = otc_pool.tile([128, NPAIR, 128], F32R, tag="otc")
                    nc.sync.dma_start(ot_c[:], OT_dram[:, :, lsl])
                    for co in range(C // 512):
                        pps = ps_c.tile([128, 512], F32, tag="pps")
                        for kd in range(NPAIR):
                            nc.tensor.matmul(
                                pps[:],
                                ot_c[:, kd, :],
                                wpT_t[:, kd, co * 512:(co + 1) * 512],
                                start=(kd == 0),
                                stop=(kd == NPAIR - 1),
                            )
                        ob = ob_pool.tile([128, 512], F32, tag="ob")
                        nc.vector.tensor_copy(ob[:], pps[:])
                        nc.sync.dma_start(outp_d[lt, :, co * 512:(co + 1) * 512], ob[:])

    nc.compile()
    return nc


def _get_nc():
    if "nc" not in _built:
        _built["nc"] = _build(None)
    return _built["nc"]


def _rope_perm():
    """Within-head row permutation: quadrant-local [evens(16) | odds(16)]."""
    perm = np.empty(64, np.int64)
    for j in range(2):
        for i in range(32):
            perm[j * 32 + i] = 2 * (j * 16 + i) if i < 16 else 2 * (j * 16 + i - 16) + 1
    return perm


def _shard_inputs(x, cos, sin, w_qkv, w_proj):
    perm = _rope_perm()
    p = np.arange(128)
    quad, i = p // 32, p % 32
    pairidx = (quad % 2) * 16 + (i % 16)
    sign = np.where(i < 16, -1.0, 1.0).astype(np.float32)
    cos4 = np.ascontiguousarray(cos[:, pairidx].T)                  # [128, L]
    sin4 = np.ascontiguousarray((sin[:, pairidx] * sign[None, :]).T)

    in_maps = []
    for c in range(NCORES):
        b, hg = c // 2, c % 2
        xT = np.ascontiguousarray(
            x[b].T.reshape(C // 128, 128, L).transpose(1, 0, 2)
        )  # [p, kc, l]

        rows = np.empty((8, 128), np.int64)
        for ft in range(8):
            t = 0 if ft < 4 else 1
            pr = ft % 4
            for fi in range(128):
                head = hg * 8 + 2 * pr + (0 if fi < 64 else 1)
                rows[ft, fi] = t * C + head * D + perm[fi % 64]
        wq = w_qkv[rows.reshape(-1)].reshape(8, 128, C // 128, 128)  # [ft, f, kc, p]
        wqkT = np.ascontiguousarray(wq.transpose(0, 3, 2, 1))        # [ft, p, kc, f]

        wv = w_qkv[2 * C + hg * 512: 2 * C + hg * 512 + 512]         # [fv, c]
        wvT = np.ascontiguousarray(wv.T.reshape(C // 128, 128, 512))  # [kc, p, fv]

        wp = w_proj[:, hg * 512: hg * 512 + 512]                     # [co, d']
        wpT = np.ascontiguousarray(
            wp.T.reshape(4, 128, C).transpose(1, 0, 2)
        )  # [p, kd, co]

        in_maps.append(
            {"xT": xT, "wqkT": wqkT, "wvT": wvT, "wpT": wpT, "cos4": cos4, "sin4": sin4}
        )
    return in_maps


def kernel(x, cos, sin, w_qkv, w_proj, b_proj, _trace=False):
    from concourse.bass_utils import run_bass_kernel_spmd

    x = np.asarray(x, dtype=np.float32)
    cos = np.asarray(cos, dtype=np.float32)
    sin = np.asarray(sin, dtype=np.float32)
    w_qkv = np.asarray(w_qkv, dtype=np.float32)
    w_proj = np.asarray(w_proj, dtype=np.float32)
    b_proj = np.asarray(b_proj, dtype=np.float32)

    nc = _get_nc()
    in_maps = _shard_inputs(x, cos, sin, w_qkv, w_proj)
    res = run_bass_kernel_spmd(
        nc, in_maps, core_ids=list(range(NCORES)), trace=_trace
    )
    if _trace:
        print("exec_time_ns:", res.exec_time_ns)
        print("trace:", res.instructions_and_trace[1] if res.instructions_and_trace else None)

    out = np.empty((B, L, C), dtype=np.float32)
    for b in range(B):
        p0 = res.results[2 * b]["outp"].reshape(L, C)
        p1 = res.results[2 * b + 1]["outp"].reshape(L, C)
        out[b] = p0 + p1
    out += b_proj[None, None, :]
    return out



# revision 48
# speedup vs baseline: 1.2565x; 1.2565x over previous
"""Multi-head attention (B=4, L=2048, C=1024, H=16, D=64) on 8 TRN2 NeuronCores.

Sharding: core c handles batch b = c//2 and head-group hg = c%2 (8 heads).
Megatron-style: w_qkv column-sharded, w_proj row-sharded; the proj all-reduce
(2 cores per batch) happens on the host during unshard.

Per-core dataflow (v3):
  - QKV-projection inputs (x, w_qkv slices) are BF16; q/k proj accumulates in
    fp32 PSUM, RoPE on DVE (host-permuted weights + quadrant-swap
    stream_shuffle), qT/kT kept fp32r.
  - V = x @ Wv (all 8 heads wide), stored BF16 with a ones column at 64 (the
    softmax-denominator trick).
  - scores: S^T[k, q] = kT.T @ qT per 128-k-chunk; two heads packed per 128
    partitions via tile_position; one PSUM bank per head.
  - softmax: exp on ScalarE over 3-bank groups (scale 1/8 fused), BF16 out;
    the larger groups amortize the per-instruction SBUF-access overhead
    (~185ns) across 1536 elements.
  - attn@V (output-stationary): O[q, d] = P^T.T @ V' per 128-q block, BF16 at
    full PE rate (ap=65/matmul). Each (head, q-block) region is a SINGLE
    sequential 16-matmul PSUM chain (start zeroes the whole bank, so chains
    NEVER interleave within a bank), ping-ponging across two 1-bank tiles
    with a copy-out to SBUF between chains; column 64 accumulates the
    denominator.
  - normalize: batched reciprocal + per-region tensor_scalar (the denominator
    sits on the partition axis after the restructure, so it is a per-partition
    scalar — no broadcast needed).
  - transpose: one PE transpose per 128-q block (both heads at once) turns
    O_n[q, d-pair] into OT[d-pair, q] BF16 via an identity matrix.
  - out-proj: out[l, :] = sum_kd OT[kd].T @ wpT[kd] in BF16; OT lives in SBUF
    (no DRAM roundtrip).
  - Scheduling: the exp stream is the pacing engine (~1.47us per 1536-element
    group). Each q-tile's slots emit only scores+exp; ALL other PE work --
    attn@V chains (2 per slot) + normalize of the previous q-tile, V-proj,
    later q/k chunks, out-proj halves -- is queued as filler quanta emitted
    ahead of anything that waits on the current exp. emit_qk is idempotent
    per chunk with just-in-time guards, so fillers are scheduling hints and
    correctness never depends on queue timing. This keeps both PE (~87%) and
    ScalarE (~96%) busy; TimelineSim 332.4us vs 417.7us baseline (1.26x).
"""

import sys

sys.path.insert(0, "/opt/trn_rl_repo")

import numpy as np

B, L, C, H, D = 4, 2048, 1024, 16, 64
NCORES = 8
QT = 512          # q-tile; score mm N=512 = one full PSUM bank
GRP = 3           # score chunk-banks per exp group (ps_sc = 2 bufs x 3 banks)
PT_BUFS = 17
QK_BUFS = 4
WQK_BUFS = 4
_built = {}


def _build(nc_mod):
    """Build the per-core Bass program (identical on all cores)."""
    import concourse.mybir as mybir
    import concourse.tile as tile
    from concourse import bacc
    from concourse.alu_op_type import AluOpType

    F32 = mybir.dt.float32
    F32R = mybir.dt.float32r
    BF16 = mybir.dt.bfloat16
    EXP = mybir.ActivationFunctionType.Exp
    MULT = AluOpType.mult
    ADD = AluOpType.add

    NKC = C // 128          # 8 contraction chunks for qkv proj
    NLT = L // 128          # 16 l-tiles (V rows, proj rows, k-chunks)
    NQT = L // QT           # q-tiles per pair
    NPAIR = 4               # head pairs per core
    FV = 512                # v features per core
    VW = 65                 # V columns incl. ones

    nc = bacc.Bacc(None, target_bir_lowering=False)

    xT_d = nc.dram_tensor("xT", [128, NKC, L], BF16, kind="ExternalInput")
    wqkT_d = nc.dram_tensor("wqkT", [8, 128, NKC, 128], BF16, kind="ExternalInput")
    wvT_d = nc.dram_tensor("wvT", [128, NKC, FV], BF16, kind="ExternalInput")
    wpT_d = nc.dram_tensor("wpT", [128, NPAIR, C], BF16, kind="ExternalInput")
    cos4_d = nc.dram_tensor("cos4", [128, L], F32, kind="ExternalInput")
    sin4_d = nc.dram_tensor("sin4", [128, L], F32, kind="ExternalInput")
    ident_d = nc.dram_tensor("ident", [128, 128], BF16, kind="ExternalInput")
    outp_d = nc.dram_tensor("outp", [NLT, 128, C], F32, kind="ExternalOutput")
    import os
    DBG = bool(os.environ.get("K_DEBUG"))
    if DBG:
        vdbg_d = nc.dram_tensor("vdbg", [128, NLT, 8, 64], BF16, kind="ExternalOutput")
        q0dbg_d = nc.dram_tensor("q0dbg", [128, L], F32, kind="ExternalOutput")
        k0dbg_d = nc.dram_tensor("k0dbg", [128, L], F32, kind="ExternalOutput")
        otdbg_d = nc.dram_tensor("otdbg", [NPAIR, 128, L], BF16, kind="ExternalOutput")

    SWAP_MASK = list(range(16, 32)) + list(range(16))

    with tile.TileContext(nc) as tc:
        import contextlib

        with contextlib.ExitStack() as outer:
            persist = outer.enter_context(tc.tile_pool(name="persist", bufs=1))

            with contextlib.ExitStack() as mid:
                xpool = mid.enter_context(tc.tile_pool(name="xT", bufs=1))
                wpool = mid.enter_context(tc.tile_pool(name="wA", bufs=WQK_BUFS))
                cpool = mid.enter_context(tc.tile_pool(name="csn", bufs=1))
                tpool = mid.enter_context(tc.tile_pool(name="tmp", bufs=2))
                qk_pool = mid.enter_context(tc.tile_pool(name="qkt", bufs=QK_BUFS))
                pt_pool = mid.enter_context(tc.tile_pool(name="pt", bufs=PT_BUFS))
                on_pool = mid.enter_context(tc.tile_pool(name="on", bufs=2))
                rec_pool = mid.enter_context(tc.tile_pool(name="rec", bufs=2))
                ob_pool = mid.enter_context(tc.tile_pool(name="ob", bufs=4))
                ps_sc = mid.enter_context(tc.tile_pool(name="ps_sc", bufs=2, space="PSUM"))
                ps_av = mid.enter_context(tc.tile_pool(name="ps_av", bufs=2, space="PSUM"))
                ps_den = mid.enter_context(tc.tile_pool(name="ps_den", bufs=1, space="PSUM"))
                ps_qp = mid.enter_context(tc.tile_pool(name="ps_qp", bufs=1, space="PSUM"))

                # ---- persistent tensors ----
                V_t = persist.tile([128, NLT, 8, 64], BF16, tag="V")
                wpT_t = persist.tile([128, NPAIR, C], BF16, tag="wp")
                ident_t = persist.tile([128, 128], BF16, tag="id")
                cos4_t = persist.tile([128, L], F32, tag="cos")
                sin4_t = persist.tile([128, L], F32, tag="sin")
                OT_ts = [
                    persist.tile([128, L], BF16, tag=f"OT{p}", name=f"OT{p}")
                    for p in range(NPAIR)
                ]

                # ---- input DMAs, batched + ordered for earliest exp start ----
                wvpool = mid.enter_context(tc.tile_pool(name="wV", bufs=1))
                wvT_t = wvpool.tile([128, NKC, FV], BF16, tag="wv")
                xT_t = xpool.tile([128, NKC, L], BF16)
                nc.sync.dma_start(xT_t[:, :, 0:512], xT_d[:, :, 0:512])

                qkT = {}
                wqk_tiles = {}

                def fetch_wqk(ft):
                    wqk = wpool.tile([128, NKC, 128], BF16, tag="wqk")
                    nc.sync.dma_start(wqk[:], wqkT_d[ft])
                    wqk_tiles[ft] = wqk
                    qkT[ft] = qk_pool.tile([128, L], F32R, tag="qkt", name=f"qkt{ft}")

                fetch_wqk(0)
                fetch_wqk(4)
                nc.sync.dma_start(cos4_t[:, 0:512], cos4_d[:, 0:512])
                nc.sync.dma_start(sin4_t[:, 0:512], sin4_d[:, 0:512])
                nc.sync.dma_start(xT_t[:, :, 512:1024], xT_d[:, :, 512:1024])
                nc.sync.dma_start(cos4_t[:, 512:2048], cos4_d[:, 512:2048])
                nc.sync.dma_start(sin4_t[:, 512:2048], sin4_d[:, 512:2048])
                nc.sync.dma_start(wvT_t[:], wvT_d[:])
                nc.sync.dma_start(xT_t[:, :, 1024:1536], xT_d[:, :, 1024:1536])
                nc.sync.dma_start(xT_t[:, :, 1536:2048], xT_d[:, :, 1536:2048])
                nc.sync.dma_start(ident_t[:], ident_d[:])
                nc.sync.dma_start(wpT_t[:], wpT_d[:])
                ones_sb = cpool.tile([128, 1], BF16, tag="ones")
                nc.vector.memset(ones_sb[:], 1.0)

                # ---- A1: V = x @ Wv for ALL pairs, one l-tile at a time ----
                def emit_a1(lt0, lt1):
                    for lt in range(lt0, lt1):
                        vps = ps_sc.tile([128, 512], F32, tag="sc", name="vps")
                        for kc in range(NKC):
                            nc.tensor.matmul(
                                vps[:],
                                xT_t[:, kc, lt * 128:(lt + 1) * 128],
                                wvT_t[:, kc, :],
                                start=(kc == 0),
                                stop=(kc == NKC - 1),
                            )
                        nc.vector.tensor_copy(V_t[:, lt, :, 0:64], vps[:])

                # ---- A2: qT/kT for one f-tile, lq-chunk range ----
                qk_done = set()

                def emit_qk(ft, lq0, lq1):
                    if ft not in qkT:
                        fetch_wqk(ft)
                    wqk, dst = wqk_tiles[ft], qkT[ft]
                    for lq in range(lq0, lq1):
                        if (ft, lq) in qk_done:
                            continue
                        qk_done.add((ft, lq))
                        qps = ps_av.tile([128, 512], F32, tag="avc", name="qps")
                        sl = slice(lq * 512, (lq + 1) * 512)
                        for kc in range(NKC):
                            nc.tensor.matmul(
                                qps[:],
                                wqk[:, kc, :],
                                xT_t[:, kc, sl],
                                start=(kc == 0),
                                stop=(kc == NKC - 1),
                            )
                        # RoPE: dst = qps*cos4 + swap(qps)*sin4s
                        shuf = tpool.tile([128, 512], F32, tag="shuf")
                        nc.vector.stream_shuffle(shuf[:], qps[:], SWAP_MASK)
                        nc.vector.tensor_tensor(dst[:, sl], qps[:], cos4_t[:, sl], op=MULT)
                        nc.vector.tensor_tensor(shuf[:], shuf[:], sin4_t[:, sl], op=MULT)
                        nc.vector.tensor_tensor(dst[:, sl], dst[:, sl], shuf[:], op=ADD)

                # prologue: just the first q/k chunks (V comes via fillers)
                emit_qk(0, 0, 1)
                emit_qk(4, 0, 1)
                emit_qk(4, 1, 2)

                # group schedule for one q-tile: [(g0, glen)]
                SUNITS = 2 * NLT  # 32 (kt, hd) units
                GROUPS = [
                    (g0, min(GRP, SUNITS - g0)) for g0 in range(0, SUNITS, GRP)
                ]

                # deferred-normalize state from the previous q-tile
                pending = {}
                # pending C l-tile chunks (pair 3 only)
                c_ready = []

                def _norm_ts(hd):
                    p = pending
                    av4, on_t, rec = p["av4"], p["on"], p["rec"]
                    for qs in range(4):
                        nc.vector.tensor_scalar(
                            on_t[:, qs, hd * 64:(hd + 1) * 64],
                            av4[:, hd, qs, :],
                            rec[:, hd, qs:qs + 1],
                            None,
                            op0=MULT,
                        )

                def norm_stage1a():
                    """reciprocal + hd0 normalize into O_n (DVE)."""
                    p = pending
                    rec = rec_pool.tile([128, 2, 4], F32, tag="rec")
                    p["rec"] = rec
                    nc.vector.reciprocal(rec[:], p["den"][:, :, :, 0:1])
                    _norm_ts(0)

                def norm_stage1b():
                    _norm_ts(1)

                def norm_stage2():
                    """PE transpose O_n -> OT (bf16) + DVE copy to SBUF.

                    Transpose scratch lives in the (already-consumed) av4
                    PSUM buffer, bitcast to bf16 — no extra bank needed."""
                    p = pending
                    av4, on_t, OT_t, qt = p["av4"], p["on"], p["ot"], p["qt"]
                    for qs in range(4):
                        tp = av4[:, 0, qs, :].bitcast(BF16)  # [128, 128] bf16
                        nc.tensor.transpose(tp, on_t[:, qs, :], ident_t[:])
                        nc.vector.tensor_copy(
                            OT_t[:, qt * QT + qs * 128: qt * QT + (qs + 1) * 128],
                            tp,
                        )

                def emit_c_half(lt, co, pool=None):
                    """out-proj for one (l-tile, 512-col half): 4 matmuls."""
                    pps = (pool or ps_qp).tile([128, 512], F32, tag="qps" if pool is None else "sc", name="pps")
                    for kd in range(NPAIR):
                        nc.tensor.matmul(
                            pps[:],
                            OT_ts[kd][:, lt * 128:(lt + 1) * 128],
                            wpT_t[:, kd, co * 512:(co + 1) * 512],
                            start=(kd == 0),
                            stop=(kd == NPAIR - 1),
                        )
                    ob = ob_pool.tile([128, 512], F32, tag="ob")
                    nc.vector.tensor_copy(ob[:], pps[:])
                    nc.sync.dma_start(outp_d[lt, :, co * 512:(co + 1) * 512], ob[:])

                # ---- phase B: attention per (pair, q-tile) ----
                def emit_av(av4, den, pr, g0, glen, pt):
                    for j in range(glen):
                        s = g0 + j
                        kt, hd = s // 2, s % 2
                        for qs in range(4):
                            lhsT = pt[:, j, qs * 128:(qs + 1) * 128]
                            nc.tensor.matmul(
                                av4[:, hd, qs, :],
                                lhsT,
                                V_t[:, kt, pr * 2 + hd, :],
                                start=(kt == 0),
                                stop=(kt == NLT - 1),
                                skip_group_check=True,
                            )
                            nc.tensor.matmul(
                                den[:, hd, qs, :],
                                lhsT,
                                ones_sb[:],
                                start=(kt == 0),
                                stop=(kt == NLT - 1),
                                skip_group_check=True,
                            )

                # PE filler queue: one small item (~<=1.7us) popped per group
                # slot so fillers never burst and starve the exp stream.
                from collections import deque

                fillers = deque()
                cfill = deque()

                av_lags = deque()  # 2-deep attn@V lag
                for pr in range(NPAIR):
                    if DBG and pr == 1:
                        nc.sync.dma_start(q0dbg_d[:].bitcast(F32R), qkT[0][:])
                        nc.sync.dma_start(k0dbg_d[:].bitcast(F32R), qkT[4][:])
                    qT_t, kT_t = qkT[pr], qkT[4 + pr]
                    OT_t = OT_ts[pr]
                    for qt in range(NQT):
                        qsl = slice(qt * QT, (qt + 1) * QT)
                        av4 = ps_av.tile([128, 2, 4, 64], F32, tag="av4")
                        den = ps_den.tile([128, 2, 4, 1], F32, tag="den")
                        on_t = on_pool.tile([128, 4, 128], BF16, tag="on")

                        # C chunks for the previous q-tile of pair 3: its
                        # OT columns complete during THIS q-tile (norm2 at
                        # gi3), so its l-tiles can be projected from gi>=5.
                        if pr == NPAIR - 1 and pending and pending["ot"] is OT_t:
                            q0 = pending["qt"]
                            for lt in range(q0 * 4, q0 * 4 + 4):
                                for co in range(2):
                                    cfill.append((lt, co))
                        # queue this q-tile's filler quanta (in-need order)
                        if pr == 0 and qt == 0:
                            # V for all pairs (2 l-tiles per quantum), with
                            # this pair's remaining k/q chunks interleaved
                            for a in range(0, 6, 2):
                                fillers.append(lambda z=a: emit_a1(z, z + 2))
                            fillers.append(lambda: emit_qk(4, 2, 3))
                            fillers.append(lambda: emit_a1(6, 8))
                            fillers.append(lambda: emit_a1(8, 10))
                            fillers.append(lambda: emit_qk(4, 3, 4))
                            fillers.append(lambda: emit_a1(10, 12))
                            fillers.append(lambda: emit_qk(0, 1, 2))
                            fillers.append(lambda: emit_a1(12, 14))
                            fillers.append(lambda: emit_a1(14, 16))
                        else:
                            if qt < NQT - 1:
                                # this pair's own next qT chunk
                                fillers.append(
                                    lambda f=pr, q=qt: emit_qk(f, q + 1, q + 2)
                                )
                            if pr < NPAIR - 1:
                                if qt < 3:
                                    fillers.append(
                                        lambda f=5 + pr, q=qt: emit_qk(f, q, q + 1)
                                    )
                                else:
                                    fillers.append(
                                        lambda f=5 + pr: emit_qk(f, 3, 4)
                                    )
                                    fillers.append(
                                        lambda f=pr + 1: emit_qk(f, 0, 1)
                                    )

                        for gi, (g0, glen) in enumerate(GROUPS):
                            sc = ps_sc.tile([128, GRP, QT], F32, tag="sc")
                            pt = pt_pool.tile([128, GRP, QT], BF16, tag="pt")
                            for j in range(glen):
                                s = g0 + j
                                kt, hd = s // 2, s % 2
                                nc.tensor.matmul(
                                    sc[:, j, :],
                                    kT_t[hd * 64:(hd + 1) * 64, kt * 128:(kt + 1) * 128],
                                    qT_t[hd * 64:(hd + 1) * 64, qsl],
                                    start=True,
                                    stop=True,
                                    tile_position=(hd * 64, 0),
                                )
                            nc.scalar.activation(
                                pt[:, 0:glen, :], sc[:, 0:glen, :], EXP,
                                scale=float(D) ** -0.5,
                            )
                            # deferred normalize from the previous q-tile,
                            # one filler quantum per slot — all emitted BEFORE
                            # the lagged attn@V pop so PE work that feeds the
                            # pipeline is never queued behind an exp wait.
                            if gi == 1 and pending:
                                norm_stage1a()
                            elif gi == 2 and pending:
                                norm_stage1b()
                            elif gi == 3 and pending:
                                norm_stage2()
                            elif gi >= 5 and cfill:
                                emit_c_half(*cfill.popleft())
                            elif fillers:
                                fillers.popleft()()
                                # pair 3 has no A2 work; drain out-proj 2/slot
                                if pr == NPAIR - 1 and fillers:
                                    fillers.popleft()()
                            # attn@V with a TWO-group lag: the PE stream only
                            # waits on exp(g-2), which is long done when the
                            # stream reaches it.
                            av_lags.append((av4, den, pr, g0, glen, pt))
                            if len(av_lags) > 2:
                                emit_av(*av_lags.popleft())
                        pending = {"av4": av4, "den": den, "on": on_t, "ot": OT_t, "qt": qt}

                # ---- tail: flush lag, last normalize, remaining out-proj ----
                while av_lags:
                    emit_av(*av_lags.popleft())
                ci = 0
                while fillers:
                    th = fillers.popleft()
                    th()
                    ci += 1
                while cfill:
                    lt, co = cfill.popleft()
                    emit_c_half(lt, co, pool=ps_sc)
                norm_stage1a()
                norm_stage1b()
                norm_stage2()
                for lt in range(12, 16):
                    for co in range(2):
                        emit_c_half(lt, co, pool=ps_sc)
                if DBG:
                    nc.sync.dma_start(vdbg_d[:], V_t[:])
                    for p in range(NPAIR):
                        nc.sync.dma_start(otdbg_d[p], OT_ts[p][:])

    nc.compile()
    return nc


def _get_nc():
    if "nc" not in _built:
        _built["nc"] = _build(None)
    return _built["nc"]


def _rope_perm():
    """Within-head row permutation: quadrant-local [evens(16) | odds(16)]."""
    perm = np.empty(64, np.int64)
    for j in range(2):
        for i in range(32):
            perm[j * 32 + i] = 2 * (j * 16 + i) if i < 16 else 2 * (j * 16 + i - 16) + 1
    return perm


def _shard_inputs(x, cos, sin, w_qkv, w_proj):
    import ml_dtypes

    bf16 = ml_dtypes.bfloat16
    perm = _rope_perm()
    p = np.arange(128)
    quad, i = p // 32, p % 32
    pairidx = (quad % 2) * 16 + (i % 16)
    sign = np.where(i < 16, -1.0, 1.0).astype(np.float32)
    cos4 = np.ascontiguousarray(cos[:, pairidx].T)                  # [128, L]
    sin4 = np.ascontiguousarray((sin[:, pairidx] * sign[None, :]).T)
    ident = np.eye(128, dtype=bf16)

    in_maps = []
    for c in range(NCORES):
        b, hg = c // 2, c % 2
        xT = np.ascontiguousarray(
            x[b].T.reshape(C // 128, 128, L).transpose(1, 0, 2)
        ).astype(bf16)  # [p, kc, l]

        rows = np.empty((8, 128), np.int64)
        for ft in range(8):
            t = 0 if ft < 4 else 1
            pr = ft % 4
            for fi in range(128):
                head = hg * 8 + 2 * pr + (0 if fi < 64 else 1)
                rows[ft, fi] = t * C + head * D + perm[fi % 64]
        wq = w_qkv[rows.reshape(-1)].reshape(8, 128, C // 128, 128)  # [ft, f, kc, p]
        wqkT = np.ascontiguousarray(wq.transpose(0, 3, 2, 1)).astype(bf16)

        wv = w_qkv[2 * C + hg * 512: 2 * C + hg * 512 + 512]         # [fv, c]
        wvT = np.ascontiguousarray(
            wv.T.reshape(C // 128, 128, 512).transpose(1, 0, 2)
        ).astype(bf16)  # [p, kc, fv]

        wp = w_proj[:, hg * 512: hg * 512 + 512]                     # [co, d']
        wpT = np.ascontiguousarray(
            wp.T.reshape(4, 128, C).transpose(1, 0, 2)
        ).astype(bf16)  # [p, kd, co]

        in_maps.append(
            {
                "xT": xT,
                "wqkT": wqkT,
                "wvT": wvT,
                "wpT": wpT,
                "cos4": cos4,
                "sin4": sin4,
                "ident": ident,
            }
        )
    return in_maps


def kernel(x, cos, sin, w_qkv, w_proj, b_proj, _trace=False):
    from concourse.bass_utils import run_bass_kernel_spmd

    x = np.asarray(x, dtype=np.float32)
    cos = np.asarray(cos, dtype=np.float32)
    sin = np.asarray(sin, dtype=np.float32)
    w_qkv = np.asarray(w_qkv, dtype=np.float32)
    w_proj = np.asarray(w_proj, dtype=np.float32)
    b_proj = np.asarray(b_proj, dtype=np.float32)

    nc = _get_nc()
    in_maps = _shard_inputs(x, cos, sin, w_qkv, w_proj)
    res = run_bass_kernel_spmd(
        nc, in_maps, core_ids=list(range(NCORES)), trace=_trace
    )
    if _trace:
        print("exec_time_ns:", res.exec_time_ns)
        print("trace:", res.instructions_and_trace[1] if res.instructions_and_trace else None)

    out = np.empty((B, L, C), dtype=np.float32)
    for b in range(B):
        p0 = res.results[2 * b]["outp"].reshape(L, C)
        p1 = res.results[2 * b + 1]["outp"].reshape(L, C)
        out[b] = p0 + p1
    out += b_proj[None, None, :]
    return out
